# revision 3
# baseline (speedup 1.0000x reference)
"""LogicLayer Trainium2 kernel v11. Core scheme (see kernel_v2/v4): fp8+fp16
hybrid gathers of x^T rows, host k-prep, DVE computes ab, PE applies
k1/k2/k3 as diag-matmuls into PSUM, ACT evicts fp8 delta, host adds k0.

v11 over v4:
- Const loads hoisted out of the rep loop and split across SP/ACT DGE
  queues (shorter pipeline fill; no rep-boundary serialization).
- Variable chunking: 15x512 + 2x256 indices -- the small final chunks
  halve the drain tail.
- The per-chunk ACT diag build moves to Pool on odd chunks (ACT is
  hotter than Pool).
- Flat [P, NCOL*BATCH] output tensor.
"""
import numpy as np

from concourse import bacc, mybir, tile
from concourse.bass_utils import run_bass_kernel_spmd

BATCH = 512
IN_DIM = 8192
OUT_DIM = 65536
N_CORES = 8
SHARD = OUT_DIM // N_CORES
P = 128
NCOL = SHARD // P  # 64

# Chunk sizes in neuron indices; must sum to SHARD and divide by 256.
CHUNKS = [512] * 15 + [256, 256]
FP16_CHUNKS = frozenset((1, 3, 5, 7, 9, 11, 13))
SUB_COLS = 2  # columns per compute sub-chunk (PSUM tile = 2 banks)

FP16 = mybir.dt.float16
FP8 = mybir.dt.float8e4
F32 = mybir.dt.float32
I16 = mybir.dt.int16

KSCALE = 16.0

OP_COEFFS = np.array([
    [0.0,  0.0,  0.0,  0.0],
    [0.0,  0.0,  0.0,  1.0],
    [0.0,  1.0,  0.0, -1.0],
    [0.0,  1.0,  0.0,  0.0],
    [0.0,  0.0,  1.0, -1.0],
    [0.0,  0.0,  1.0,  0.0],
    [0.0,  1.0,  1.0, -2.0],
    [0.0,  1.0,  1.0, -1.0],
    [1.0, -1.0, -1.0,  1.0],
    [1.0, -1.0, -1.0,  2.0],
    [1.0,  0.0, -1.0,  0.0],
    [1.0,  0.0, -1.0,  1.0],
    [1.0, -1.0,  0.0,  0.0],
    [1.0, -1.0,  0.0,  1.0],
    [1.0,  0.0,  0.0, -1.0],
    [1.0,  0.0,  0.0,  0.0],
], dtype=np.float32)

IDX_W = SHARD // 16  # total wrapped index columns per stream


def build_program(n_reps: int = 1):
    nc = bacc.Bacc("TRN2", target_bir_lowering=False, debug=False,
                   num_devices=N_CORES, num_swdge_queues=2)

    xt8 = nc.dram_tensor("xt8", [IN_DIM, BATCH], FP8, kind="ExternalInput")
    xt16 = nc.dram_tensor("xt16", [IN_DIM, BATCH], FP16, kind="ExternalInput")
    kk = nc.dram_tensor("kk", [P, 3 * NCOL], F32, kind="ExternalInput")
    eye = nc.dram_tensor("eye", [P, P], FP16, kind="ExternalInput")
    idxa = nc.dram_tensor("idxa", [P, IDX_W], I16, kind="ExternalInput")
    idxb = nc.dram_tensor("idxb", [P, IDX_W], I16, kind="ExternalInput")
    out = nc.dram_tensor("out", [P, NCOL * BATCH], FP8, kind="ExternalOutput")

    with tile.TileContext(nc) as tc:
        with tc.tile_pool(name="const", bufs=1) as cpool, \
             tc.tile_pool(name="work", bufs=3) as pool, \
             tc.tile_pool(name="diags", bufs=6) as dpool, \
             tc.tile_pool(name="psum", bufs=4, space="PSUM") as ppool:
            # Consts loaded once, shared by all reps. idx feeds the gathers
            # (SP queue); kk/eye feed the diag builds (ACT queue, parallel).
            eye_sb = cpool.tile([P, P], FP16, tag="eye_sb")
            kk_sb = cpool.tile([P, 3, NCOL], F32, tag="kk_sb")
            ia_all = cpool.tile([P, IDX_W], I16, tag="ia")
            ib_all = cpool.tile([P, IDX_W], I16, tag="ib")
            nc.sync.dma_start(out=ia_all[:], in_=idxa[:])
            nc.sync.dma_start(out=ib_all[:], in_=idxb[:])
            nc.scalar.dma_start(out=kk_sb[:].rearrange("p t c -> p (t c)"), in_=kk[:])
            nc.scalar.dma_start(out=eye_sb[:], in_=eye[:])

            for _rep in range(n_reps):
                iw = 0   # wrapped-index column offset
                c0 = 0   # global column offset
                for chunk, ch_idx in enumerate(CHUNKS):
                    cols = ch_idx // P
                    hi = chunk in FP16_CHUNKS
                    xt, gdt, gtag = (xt16, FP16, "g16") if hi else (xt8, FP8, "g8")
                    csl = slice(iw, iw + ch_idx // 16)

                    av = pool.tile([P, cols, BATCH], gdt, tag=f"a{gtag}{cols}")
                    bv = pool.tile([P, cols, BATCH], gdt, tag=f"b{gtag}{cols}")
                    nc.gpsimd.dma_gather(
                        out_ap=av[:], in_ap=xt[:], idxs_ap=ia_all[:, csl],
                        num_idxs=ch_idx, num_idxs_reg=ch_idx, elem_size=BATCH,
                        queue_num=0)
                    nc.gpsimd.dma_gather(
                        out_ap=bv[:], in_ap=xt[:], idxs_ap=ib_all[:, csl],
                        num_idxs=ch_idx, num_idxs_reg=ch_idx, elem_size=BATCH,
                        queue_num=1)

                    o8 = pool.tile([P, cols * BATCH], FP8, tag=f"o8{cols}")
                    for sub in range(cols // SUB_COLS):
                        dg = dpool.tile([P, SUB_COLS, 3, P], FP16, tag="dg")
                        for gg in range(SUB_COLS):
                            g = sub * SUB_COLS + gg
                            col = c0 + g
                            for t in range(3):
                                ksl = kk_sb[:, t, col:col + 1]
                                dsl = dg[:, gg, t, :]
                                if gg == 0 and t == 2:
                                    if chunk % 2 == 0:
                                        nc.scalar.mul(dsl, eye_sb[:], ksl)
                                    else:
                                        nc.gpsimd.tensor_scalar(
                                            out=dsl, in0=eye_sb[:], scalar1=ksl,
                                            scalar2=None, op0=mybir.AluOpType.mult)
                                else:
                                    nc.vector.tensor_scalar(
                                        out=dsl, in0=eye_sb[:], scalar1=ksl,
                                        scalar2=None, op0=mybir.AluOpType.mult)

                        ab = pool.tile([P, SUB_COLS, BATCH], FP16, tag="ab")
                        sb = slice(sub * SUB_COLS, (sub + 1) * SUB_COLS)
                        nc.vector.tensor_tensor(out=ab[:], in0=av[:, sb], in1=bv[:, sb],
                                                op=mybir.AluOpType.mult)

                        ps = ppool.tile([P, SUB_COLS * BATCH], F32, tag="ps")
                        for gg in range(SUB_COLS):
                            g = sub * SUB_COLS + gg
                            sl = ps[:, gg * BATCH:(gg + 1) * BATCH]
                            nc.tensor.matmul(sl, dg[:, gg, 0, :], av[:, g, :],
                                             start=True, stop=False)
                            nc.tensor.matmul(sl, dg[:, gg, 1, :], bv[:, g, :],
                                             start=False, stop=False)
                        for gg in range(SUB_COLS):
                            sl = ps[:, gg * BATCH:(gg + 1) * BATCH]
                            nc.tensor.matmul(sl, dg[:, gg, 2, :], ab[:, gg, :],
                                             start=False, stop=True)
                        nc.scalar.activation(
                            o8[:, sub * SUB_COLS * BATCH:(sub + 1) * SUB_COLS * BATCH],
                            ps[:], mybir.ActivationFunctionType.Copy)
                    nc.sync.dma_start(
                        out=out[:, c0 * BATCH:(c0 + cols) * BATCH], in_=o8[:])
                    iw += ch_idx // 16
                    c0 += cols
    nc.compile()
    return nc


def host_k(weights):
    w = weights.astype(np.float64)
    e = np.exp(w - w.max(axis=1, keepdims=True))
    p = e / e.sum(axis=1, keepdims=True)
    return (p @ OP_COEFFS.astype(np.float64)).astype(np.float32)


def _wrap_idx(stream):
    """[SHARD] index stream -> [P, IDX_W] wrapped i16 (16-partition wrap,
    chunk blocks concatenated, tiled to 128 partitions)."""
    blocks = []
    off = 0
    for ch_idx in CHUNKS:
        blk = stream[off:off + ch_idx].reshape(ch_idx // 16, 16).T  # [16, w]
        blocks.append(blk)
        off += ch_idx
    w = np.concatenate(blocks, axis=1)  # [16, IDX_W]
    return np.ascontiguousarray(np.tile(w, (8, 1)))  # [128, IDX_W]


def make_in_maps(x, weights, connections):
    xt = np.ascontiguousarray(x.T)
    xt8 = xt.astype(mybir.dt.np(FP8))
    xt16 = xt.astype(np.float16)
    eye_np = np.eye(P, dtype=np.float16)

    k = host_k(np.asarray(weights))
    k123 = (k[:, 1:4] * KSCALE).astype(np.float32)

    in_maps = []
    for c in range(N_CORES):
        base = c * SHARD
        ks = k123[base:base + SHARD]
        kk_dev = np.ascontiguousarray(
            ks.reshape(NCOL, P, 3).transpose(1, 2, 0).reshape(P, 3 * NCOL))

        conn = connections[base:base + SHARD].astype(np.int16)
        in_maps.append({
            "xt8": xt8, "xt16": xt16, "kk": kk_dev, "eye": eye_np,
            "idxa": _wrap_idx(conn[:, 0]), "idxb": _wrap_idx(conn[:, 1]),
        })
    return in_maps


def assemble_output(results, k0):
    shards = []
    for c in range(N_CORES):
        o = np.asarray(results[c]["out"]).astype(np.float32)
        o = o.reshape(P, NCOL, BATCH).transpose(1, 0, 2)  # [col, p, batch]
        shards.append(o.reshape(SHARD, BATCH))
    delta = np.concatenate(shards, axis=0)
    full = delta * (1.0 / KSCALE) + k0[:, None]
    return np.ascontiguousarray(full.T)


_CACHED_NC = None


def kernel(x, weights, connections):
    global _CACHED_NC
    if _CACHED_NC is None:
        _CACHED_NC = build_program()
    x = np.asarray(x)
    weights = np.asarray(weights)
    connections = np.asarray(connections)
    in_maps = make_in_maps(x, weights, connections)
    k0 = host_k(weights)[:, 0]
    last_err = None
    for _attempt in range(3):
        try:
            res = run_bass_kernel_spmd(_CACHED_NC, in_maps, list(range(N_CORES)))
            return assemble_output(res.results, k0)
        except Exception as e:
            last_err = e
    raise last_err


if __name__ == "__main__":
    rng = np.random.default_rng(0)
    x = rng.random((BATCH, IN_DIM), dtype=np.float32)
    weights = (rng.standard_normal((OUT_DIM, 16)) * 0.1).astype(np.float32)
    connections = rng.integers(0, IN_DIM, size=(OUT_DIM, 2), dtype=np.int64)
    out = kernel(x, weights, connections)
    print("out", out.shape, out.dtype)
